# revision 1
# baseline (speedup 1.0000x reference)
"""Dynamic depthwise 3x3 conv (per-pixel weights) on 8 Trainium2 NeuronCores.

Problem:
  x:            [4, 64, 256, 256]  f32
  conv_weights: [4, 576, 256, 256] f32  (= [4, 64ch * 9tap, 256, 256])
  out[n,c,h,w] = sum_k w[n, c*9+k, h, w] * xpad[n, c, h+ki, w+kj],  k=(ki,kj) row-major

Sharding: pure data parallel over (batch n, H-half) -> 8 shards.

On-core layout: partition p = hb*64 + c (hb in {0,1} picks a 64-row block of
the core's 128 output rows, c the channel). x is stored UNPADDED in W
(rows of 256, H-padded on the host), so the flattened (h, w) index j is
contiguous and a single custom-DVE segmented-MAC instruction covers a whole
Rh-row tile for one kernel-row dh:

    tgt[p, j] = sum_dw w[p, dh, j, dw] * x[p, j + dh*256 + dw - 1]

Width-edge taps (wd=0,dw=0 and wd=255,dw=2) would wrap into the neighbouring
row; in the reference those taps multiply zero-padding, so the host repack
zeroes those weight entries and the wrap contributions vanish exactly.

conv_weights are repacked host-side to [T, 128, dh, (h,w), dw] so each
DMA is one sequential HBM stream (~27 GB/s/engine descriptors) and the MAC's
dw-segments are innermost. The custom DVE op (segmac.py) does the 3-tap
dot product per output element in one pass with a per-segment scan reset.
"""

import sys

sys.path.insert(0, "/opt/trn_rl_repo")

import numpy as np

import concourse.bass as bass
import concourse.bacc as bacc
import concourse.tile as tile
from concourse import mybir
from concourse.bass_utils import run_bass_kernel_spmd


# ---------------------------------------------------------------------------
# Custom DVE op: segmented multiply-accumulate (dot-KW per output element).
#   out[p, s] = sum_n in0[p, s, n] * in1[p, s, n]
# A scan(ADD, Src0*Src1) whose accumulator resets at each SUB_DIM_DONE (the
# per-page "per_subdim" STEP state the HW supports but the stock Spec DSL does
# not expose); the out AP uses a step-0 inner dim so the last (complete)
# partial of each segment is what lands at out[p, s]. Streams both tensors at
# 1 elem/lane/cycle: a 3-tap dot product costs 3 input cycles, no reduction
# passes.
# ---------------------------------------------------------------------------

from dataclasses import dataclass

import concourse.dve_spec as dve_spec
import concourse.dve_ops as dve_ops
from concourse.dve_spec import AluOp, Spec, Src0, Src1
from concourse.dve_uop import DveOpSpec

OP_NAME = "SEG_MAC_ANT"



@dataclass(frozen=True)
class _ResetScan(dve_spec.Scan):
    """scan() that re-seeds from `init` at each SUB_DIM_DONE."""


def _patched_scan_overrides(scans, node_stage):
    seed, step = {}, {}
    for scan in scans:
        d = node_stage[scan]
        init = dve_spec._scan_init(scan)
        seed[d] = dve_spec._node_as_stage(init)
        if isinstance(scan, _ResetScan):
            # Page boundary: restart the fold — d = init op expr (the
            # "per_subdim" STEP variant from the HW state-machine table).
            step[d] = dve_spec._Stage(scan.op, init, scan.expr)
        elif scan._subdim_step is not None:
            step[d] = dve_spec._Stage(
                scan.op, dve_spec.AluInp.CURR_ALU_OUT, scan._subdim_step
            )
    return seed, step


def _segmac_ref(in0, in1, c0, c1, c2):
    # CoreSim reference: per-segment inclusive prefix of the products.
    return np.cumsum(
        np.asarray(in0, np.float32) * np.asarray(in1, np.float32),
        axis=-1,
        dtype=np.float32,
    )


def get_segmac_op():
    """Build + register the op (idempotent). Returns the DveOp."""
    existing = getattr(dve_ops, "_ANT_SEG_MAC", None)
    if existing is not None:
        return existing

    dve_spec._scan_overrides = _patched_scan_overrides

    body = _ResetScan(AluOp.ADD, Src0 * Src1)
    spec = Spec(body=body, reference=_segmac_ref)

    shas = {}
    for ver in ("v3", "v4"):
        uops = dve_spec.lower(spec, ver=ver)
        shas[ver] = DveOpSpec(name=OP_NAME, uops=uops, rd1_en=True).sha(ver)

    op = dve_ops.DveOp(OP_NAME, spec, subdim=True, uops_sha=shas)
    dve_ops.OPS.append(op)
    dve_ops._SUB_OPCODE_FOR_NAME[OP_NAME] = (
        dve_ops._CUSTOM_DVE_ROW_BASE + len(dve_ops.OPS) - 1
    )
    dve_ops.CUSTOM_DVE_SPECS[OP_NAME] = spec
    assert dve_ops._SUB_OPCODE_FOR_NAME[OP_NAME] < 0x20
    dve_ops._ANT_SEG_MAC = op
    return op


def window_ap(sl, dims):
    """Build an AP over `sl`'s tensor/offset with explicit free dims
    [[step, count], ...] (partition dim copied from sl)."""
    import bass_rust

    return bass_rust.AP(
        sl.tensor,
        sl.offset,
        [list(sl.ap[0])] + [list(d) for d in dims],
        sl.const_val,
        sl.runtime_checks,
        sl.dep_tracking_offset,
    )


N, C, H, W = 4, 64, 256, 256
KW = 3
NCORES = 8
HH = H // 2          # rows per core
RB = HH // 2         # rows per partition block (64)
Rh = 4               # rows per h-tile
T = RB // Rh         # h-tiles per core
NXT = 4              # resident x tiles per core
XB = RB // NXT       # local output rows covered per x tile (16)
XR = XB + 2          # rows per resident x tile incl halo
XF = XR * W + 2      # x tile free elems incl 1 zero guard at each end
J = Rh * W           # flattened (h, w) positions per tile
WF = KW * KW * J     # w tile free elems
F32 = mybir.dt.float32

_CACHE = {}


def _build():
    op = get_segmac_op()
    nc = bacc.Bacc("TRN2", target_bir_lowering=False, debug=False, num_devices=NCORES)
    x_in = nc.dram_tensor("x", [NXT, 128, XF], F32, kind="ExternalInput")
    w_in = nc.dram_tensor("w", [T, 128, WF], F32, kind="ExternalInput")
    y_out = nc.dram_tensor("y", [T, 128, J], F32, kind="ExternalOutput")

    with tile.TileContext(nc) as tc:
        with (
            tc.tile_pool(name="xp", bufs=1) as xpool,
            tc.tile_pool(name="wp", bufs=2) as wpool,
            tc.tile_pool(name="op", bufs=3) as opool,
            tc.tile_pool(name="pa", bufs=1) as papool,
            tc.tile_pool(name="pb", bufs=1) as pbpool,
        ):
            # x stays resident: NXT tiles, each covering XB output rows
            # (+2 halo rows) per partition block, loaded once.
            xtiles = []
            for s in range(NXT):
                xt = xpool.tile([128, XF], F32, tag=f"x{s}")
                nc.scalar.dma_start(out=xt[:], in_=x_in[s])
                xtiles.append(xt)

            for t in range(T):
                wt = wpool.tile([128, WF], F32)
                # 3 chunked loads (one per dh group) so the first MAC can
                # start before the whole tile lands.
                for dh in range(KW):
                    c0 = dh * KW * J
                    nc.sync.dma_start(
                        out=wt[:, c0:c0 + KW * J],
                        in_=w_in[t, :, c0:c0 + KW * J],
                    )

                xt = xtiles[t * Rh // XB]
                rbase = t * Rh - (t * Rh // XB) * XB

                ot = opool.tile([128, J], F32)
                pa = papool.tile([128, J], F32)
                pb = pbpool.tile([128, J], F32)
                # one whole-tile segmented MAC per kernel row dh:
                #   tgt[p, j] = sum_dw w[dh, j, dw] * x[(rbase+dh)*W + j + dw - 1]
                # (x AP offset: the +1 guard shift and -1 dw base cancel)
                for dh, tgt in ((0, ot), (1, pa), (2, pb)):
                    w_sl = wt[:, dh * KW * J:(dh + 1) * KW * J]
                    x_sl = xt[:, (rbase + dh) * W:(rbase + dh) * W + J + 2]
                    nc.vector._custom_dve(
                        op,
                        out=window_ap(tgt[:, 0:J], [[1, J], [0, KW]]),
                        in0=window_ap(w_sl, [[KW, J], [1, KW]]),
                        in1=window_ap(x_sl, [[1, J], [1, KW]]),
                    )
                nc.vector.tensor_add(ot[:], ot[:], pa[:])
                nc.vector.tensor_add(ot[:], ot[:], pb[:])

                nc.scalar.dma_start(out=y_out[t], in_=ot[:])
    nc.compile()
    return nc


def _get_nc():
    if "nc" not in _CACHE:
        _CACHE["nc"] = _build()
    return _CACHE["nc"]


def _pack_core(xh_n: np.ndarray, w5_n: np.ndarray, hf: int):
    """Repack one core's shard into per-tile-contiguous DMA blocks.

    xh_n: [C, H+2, W] H-padded x for batch n; w5_n: [C, 9, H, W].
    Returns x_blocks [NXT, 128, XF], w_blocks [T, 128, WF].
    """
    xc = xh_n[:, hf * HH:hf * HH + HH + 2, :]          # [C, HH+2, W]
    wc = w5_n[:, :, hf * HH:(hf + 1) * HH, :]          # [C, 9, HH, W]

    xb = np.zeros((NXT, 2, C, XR * W + 2), dtype=np.float32)
    for s in range(NXT):
        for hb in range(2):
            r0 = hb * RB + s * XB
            xb[s, hb, :, 1:-1] = xc[:, r0:r0 + XR, :].reshape(C, XR * W)
    # w: [C, (dh, dw), (hb, t, r), wd] -> [t, (hb, c), dh, (r, wd), dw]
    wb = (
        wc.reshape(C, KW, KW, 2, T, Rh, W)
        .transpose(4, 3, 0, 1, 5, 6, 2)
        .copy()
    )  # [T, hb, C, dh, r, wd, dw]
    # width-edge taps multiply zero padding in the reference -> zero them
    wb[:, :, :, :, :, 0, 0] = 0.0
    wb[:, :, :, :, :, W - 1, KW - 1] = 0.0
    return (
        xb.reshape(NXT, 128, XF),
        np.ascontiguousarray(wb.reshape(T, 128, WF)),
    )


def _make_in_maps(x: np.ndarray, conv_weights: np.ndarray):
    x = np.asarray(x, dtype=np.float32)
    w5 = np.asarray(conv_weights, dtype=np.float32).reshape(N, C, KW * KW, H, W)
    xh = np.pad(x, ((0, 0), (0, 0), (1, 1), (0, 0)))

    in_maps = []
    for i in range(NCORES):
        n, hf = divmod(i, 2)
        xb, wb = _pack_core(xh[n], w5[n], hf)
        in_maps.append({"x": xb, "w": wb})
    return in_maps


def kernel(x: np.ndarray, conv_weights: np.ndarray) -> np.ndarray:
    nc = _get_nc()
    in_maps = _make_in_maps(x, conv_weights)
    res = run_bass_kernel_spmd(nc, in_maps, list(range(NCORES)))
    out = np.empty((N, C, H, W), dtype=np.float32)
    for i in range(NCORES):
        n, hf = divmod(i, 2)
        yb = res.results[i]["y"].reshape(T, 2, C, Rh, W)
        # invert: out rows h = hf*HH + hb*RB + t*Rh + h_sub
        oc = yb.transpose(2, 1, 0, 3, 4).reshape(C, HH, W)
        out[n, :, hf * HH:(hf + 1) * HH, :] = oc
    return out



# revision 4
# speedup vs baseline: 1.6520x; 1.6520x over previous
"""Dynamic depthwise 3x3 conv (per-pixel weights) on 8 Trainium2 NeuronCores.

Problem:
  x:            [4, 64, 256, 256]  f32
  conv_weights: [4, 576, 256, 256] f32  (= [4, 64ch * 9tap, 256, 256])
  out[n,c,h,w] = sum_k w[n, c*9+k, h, w] * xpad[n, c, h+ki, w+kj],  k=(ki,kj) row-major

Sharding: pure data parallel over (batch n, H-half) -> 8 shards.

v2 (bf16 + column-interleaved x, single-pass segmac):
  All streams are bf16 (tolerance is 2e-2; bf16 adds ~2e-3), halving HBM
  traffic. x is repacked host-side into a column-interleaved form
      xI[3*(r*W + u) + dh] = xpad[r + dh, u]     (per output row r)
  so the 9 taps of output (r, c) are the single stride-1 run
  xI[3c .. 3c+8] (t = 3*dw + dh), and ONE segmented-MAC DVE instruction
  per tile computes the whole 3x3 conv:
      out[p, j] = sum_{t<9} w[p, 9j+t] * xI[p, 3j+t]
  (w repacked host-side to t-order with width-edge taps zeroed). No
  partial adds, 3x fewer DVE instructions than the 3-pass variant.
"""

import sys

sys.path.insert(0, "/opt/trn_rl_repo")

import numpy as np
import ml_dtypes

import concourse.bass as bass
import concourse.bacc as bacc
import concourse.tile as tile
from concourse import mybir
from concourse.bass_utils import run_bass_kernel_spmd


# ---------------------------------------------------------------------------
# Custom DVE op: segmented multiply-accumulate (dot-9 per output element).
#   out[p, s] = sum_n in0[p, s, n] * in1[p, s, n]
# A scan(ADD, Src0*Src1) whose accumulator resets at each SUB_DIM_DONE; the
# out AP uses a step-0 inner dim so the last (complete) partial of each
# segment is what lands at out[p, s].
# ---------------------------------------------------------------------------

from dataclasses import dataclass

import concourse.dve_spec as dve_spec
import concourse.dve_ops as dve_ops
from concourse.dve_spec import AluOp, Spec, Src0, Src1
from concourse.dve_uop import DveOpSpec

OP_NAME = "SEG_MAC_ANT"


@dataclass(frozen=True)
class _ResetScan(dve_spec.Scan):
    """scan() that re-seeds from `init` at each SUB_DIM_DONE."""


def _patched_scan_overrides(scans, node_stage):
    seed, step = {}, {}
    for scan in scans:
        d = node_stage[scan]
        init = dve_spec._scan_init(scan)
        seed[d] = dve_spec._node_as_stage(init)
        if isinstance(scan, _ResetScan):
            # Page boundary: restart the fold — d = init op expr (the
            # "per_subdim" STEP variant from the HW state-machine table).
            step[d] = dve_spec._Stage(scan.op, init, scan.expr)
        elif scan._subdim_step is not None:
            step[d] = dve_spec._Stage(
                scan.op, dve_spec.AluInp.CURR_ALU_OUT, scan._subdim_step
            )
    return seed, step


def _segmac_ref(in0, in1, c0, c1, c2):
    # CoreSim reference: per-segment inclusive prefix of the products.
    return np.cumsum(
        np.asarray(in0, np.float32) * np.asarray(in1, np.float32),
        axis=-1,
        dtype=np.float32,
    )


def get_segmac_op():
    """Build + register the op (idempotent). Returns the DveOp."""
    existing = getattr(dve_ops, "_ANT_SEG_MAC", None)
    if existing is not None:
        return existing

    dve_spec._scan_overrides = _patched_scan_overrides

    body = _ResetScan(AluOp.ADD, Src0 * Src1)
    spec = Spec(body=body, reference=_segmac_ref)

    shas = {}
    for ver in ("v3", "v4"):
        uops = dve_spec.lower(spec, ver=ver)
        shas[ver] = DveOpSpec(name=OP_NAME, uops=uops, rd1_en=True).sha(ver)

    op = dve_ops.DveOp(OP_NAME, spec, subdim=True, uops_sha=shas)
    dve_ops.OPS.append(op)
    dve_ops._SUB_OPCODE_FOR_NAME[OP_NAME] = (
        dve_ops._CUSTOM_DVE_ROW_BASE + len(dve_ops.OPS) - 1
    )
    dve_ops.CUSTOM_DVE_SPECS[OP_NAME] = spec
    assert dve_ops._SUB_OPCODE_FOR_NAME[OP_NAME] < 0x20
    dve_ops._ANT_SEG_MAC = op
    return op


def window_ap(sl, dims):
    """Build an AP over `sl`'s tensor/offset with explicit free dims
    [[step, count], ...] (partition dim copied from sl)."""
    import bass_rust

    return bass_rust.AP(
        sl.tensor,
        sl.offset,
        [list(sl.ap[0])] + [list(d) for d in dims],
        sl.const_val,
        sl.runtime_checks,
        sl.dep_tracking_offset,
    )


N, C, H, W = 4, 64, 256, 256
KW = 3
NT = KW * KW          # taps per output
NCORES = 8
HH = H // 2           # rows per core
RB = HH // 2          # rows per partition block (64)
Rh = 4                # rows per h-tile
T = RB // Rh          # h-tiles per core (16)
J = Rh * W            # flattened (h, w) positions per tile (1024)
WF = NT * J           # w tile free elems (9216)
XI = 3 * J + 8        # interleaved-x tile free elems (3 guard + 3J + tail)
BF16 = mybir.dt.bfloat16

_CACHE = {}


def _build():
    op = get_segmac_op()
    nc = bacc.Bacc("TRN2", target_bir_lowering=False, debug=False, num_devices=NCORES)
    x_in = nc.dram_tensor("x", [T, 128, XI], BF16, kind="ExternalInput")
    w_in = nc.dram_tensor("w", [T, 128, WF], BF16, kind="ExternalInput")
    y_out = nc.dram_tensor("y", [T, 128, J], BF16, kind="ExternalOutput")

    with tile.TileContext(nc) as tc:
        with (
            tc.tile_pool(name="xp", bufs=2) as xpool,
            tc.tile_pool(name="wp", bufs=2) as wpool,
            tc.tile_pool(name="op", bufs=3) as opool,
        ):
            for t in range(T):
                xt = xpool.tile([128, XI], BF16)
                nc.scalar.dma_start(out=xt[:], in_=x_in[t])
                wt = wpool.tile([128, WF], BF16)
                # chunked loads so the MAC can start before the tile lands
                nchunk = 3
                for ci in range(nchunk):
                    c0 = ci * (WF // nchunk)
                    c1 = (ci + 1) * (WF // nchunk)
                    nc.sync.dma_start(
                        out=wt[:, c0:c1],
                        in_=w_in[t, :, c0:c1],
                    )

                ot = opool.tile([128, J], BF16)
                # single whole-tile segmented MAC:
                #   ot[p, j] = sum_t w[p, 9j+t] * xI[p, 3j+t]
                nc.vector._custom_dve(
                    op,
                    out=window_ap(ot[:, 0:J], [[1, J], [0, NT]]),
                    in0=window_ap(wt[:, 0:WF], [[NT, J], [1, NT]]),
                    in1=window_ap(xt[:, 0:3 * J + 6], [[3, J], [1, NT]]),
                )

                nc.scalar.dma_start(out=y_out[t], in_=ot[:])
    nc.compile()
    return nc


def _get_nc():
    if "nc" not in _CACHE:
        _CACHE["nc"] = _build()
    return _CACHE["nc"]


def _pack_core(xh_n: np.ndarray, w5_n: np.ndarray, hf: int):
    """Repack one core's shard into per-tile-contiguous bf16 DMA blocks.

    xh_n: [C, H+2, W] H-padded x for batch n; w5_n: [C, 9, H, W].
    Returns xI [T, 128, XI], wb [T, 128, WF] (both bf16).
    """
    xc = xh_n[:, hf * HH:hf * HH + HH + 2, :]          # [C, HH+2, W]
    wc = w5_n[:, :, hf * HH:(hf + 1) * HH, :]          # [C, 9, HH, W]

    # --- interleaved x: xI[t, (hb,c), 3*(r*W+u) + dh] = xc[c, hb*RB + t*Rh + r + dh, u]
    # sliding rows via stride tricks: xs[c, R, r, u] with R = row base
    xs = np.lib.stride_tricks.sliding_window_view(xc, 3, axis=1)
    # xs: [C, HH, W, 3] where xs[c, R, u, dh] = xc[c, R + dh, u]
    # output row R = hb*RB + t*Rh + r  ->  [C, 2, T, Rh, W, 3]
    xsr = xs.reshape(C, 2, T, Rh, W, 3)
    # -> [T, hb, C, Rh, W, 3]
    xI3 = np.ascontiguousarray(
        xsr.transpose(2, 1, 0, 3, 4, 5), dtype=np.float32
    ).reshape(T, 128, 3 * J)
    # layout: [3 front guard zeros][3J data][tail guard]; window for output
    # j reads elements 3j .. 3j+8 (tap t = 3*dw + dh at 3j + t - 3 + FRONT).
    xI = np.zeros((T, 128, XI), dtype=ml_dtypes.bfloat16)
    xI[:, :, 3:3 + 3 * J] = xI3.astype(ml_dtypes.bfloat16)

    # --- w stream: wb[t, (hb,c), 9*(r*W+u) + t'] = wc[c, (t'%3)*3 + t'//3, row, u]
    # i.e. per output, taps in t' = 3*dw + dh order.
    wcc = wc.reshape(C, KW, KW, 2, T, Rh, W)           # [C, dh, dw, hb, t, r, u]
    # -> [t, hb, C, r, u, dw, dh]
    wbt = wcc.transpose(4, 3, 0, 5, 6, 2, 1).copy()
    # width-edge taps multiply zero padding in the reference -> zero them
    wbt[:, :, :, :, 0, 0, :] = 0.0       # u = 0,   dw = 0
    wbt[:, :, :, :, W - 1, KW - 1, :] = 0.0  # u = W-1, dw = 2
    wb = wbt.reshape(T, 128, WF).astype(ml_dtypes.bfloat16)
    return xI, np.ascontiguousarray(wb)


def _make_in_maps(x: np.ndarray, conv_weights: np.ndarray):
    x = np.asarray(x, dtype=np.float32)
    w5 = np.asarray(conv_weights, dtype=np.float32).reshape(N, C, KW * KW, H, W)
    xh = np.pad(x, ((0, 0), (0, 0), (1, 1), (0, 0)))

    in_maps = []
    for i in range(NCORES):
        n, hf = divmod(i, 2)
        xb, wbk = _pack_core(xh[n], w5[n], hf)
        in_maps.append({"x": xb, "w": wbk})
    return in_maps


def kernel(x: np.ndarray, conv_weights: np.ndarray) -> np.ndarray:
    nc = _get_nc()
    in_maps = _make_in_maps(x, conv_weights)
    res = run_bass_kernel_spmd(nc, in_maps, list(range(NCORES)))
    out = np.empty((N, C, H, W), dtype=np.float32)
    for i in range(NCORES):
        n, hf = divmod(i, 2)
        yb = np.asarray(res.results[i]["y"], dtype=np.float32).reshape(
            T, 2, C, Rh, W
        )
        # invert: out rows h = hf*HH + hb*RB + t*Rh + r
        oc = yb.transpose(2, 1, 0, 3, 4).reshape(C, HH, W)
        out[n, :, hf * HH:(hf + 1) * HH, :] = oc
    return out
